# revision 10
# baseline (speedup 1.0000x reference)
"""Trainium2 Bass kernel for nn_Att_mlp_softmax (GNN message passing).

Reference computation:
    e = relu(h @ W1 + b1) @ W2 + b2                       # [N, 1] per-node score
    att = softmax(where(G > 0, e.T broadcast, -9e15))     # row-wise over neighbors
    out = (G.sum(-1))[:, None] * (att @ h)                # degree-rescaled aggregation

Because the pre-softmax score of entry (i, j) depends only on column j, the
masked softmax collapses algebraically:
    att[i, j] = G[i, j] * w[j] / sum_j G[i, j] * w[j],  w = exp(e + ESHIFT)
so with H' = [w * h | w | 1] (N x 130):
    Y = G @ H'
    out = Y[:, 129] * Y[:, :128] / Y[:, 128]
One big [N, N] x [N, 130] matmul replaces the N^2 softmax entirely.

Precision/perf: the harness gate is rel_err < 2e-2; a full single-bf16 pipeline
measures ~4e-3 in numpy emulation, so no hi/lo splitting anywhere.  G is an
exact 0/1 mask streamed as fp8e4 (quarter the fp32 HBM traffic) and used
directly as the matmul stationary operand against bf16 moving data (mixed
dtypes are legal; cost keys on the moving dtype).  H' chunks are built
just-in-time (1 big DVE op each) so the build overlaps the matmul stream.

Distribution: G is row-sharded across 8 NeuronCores (1024 rows each); h and the
MLP weights are replicated.  Each core's G shard is laid out [128, JC, ROWS]
(contraction-position major) so every DMA line is 8 KB contiguous.  h is passed
twice in bf16: d-major (hT, the MLP moving operand) and chunk-major (hc, for
the H' build).  No collectives needed.
"""

import numpy as np

N = 8192
D = 128
HID = 64
N_CORES = 8
ROWS = N // N_CORES          # 1024 output rows per core
JC = N // 128                # 64 contraction chunks of 128
GRP = 8                      # contraction chunks per G DMA (1 MB transfers)
ESHIFT = -1.0                # exp(e - 1): cancels exactly in the ratio, keeps
                             # w (and later fp8 w*h) in range
WARM = 4                     # PE clock-ramp dummy matmuls
DR = False                   # DoubleRow fp8 main loop (2 chunks / matmul)

NCOL = 259 if DR else 130    # [wh_hi|wh_lo|w_hi|w_lo|1] fp8 / [wh|w|1] bf16

_cache = {}


def _install_axon_hooks_shim():
    """Provide antenv.axon_hooks if the image lacks it (trn_boot step 6).

    concourse.bass_utils imports it unconditionally when BASS_TRACE is set;
    without the shim that import crashes instead of degrading.
    """
    import contextlib
    import ctypes
    import sys
    import types

    try:
        import antenv.axon_hooks  # noqa: F401
        return
    except ImportError:
        pass

    so_path = "/opt/axon/libaxon_pjrt.so"

    def _make_hook():
        try:
            lib = ctypes.CDLL(so_path)
        except OSError:
            return None
        if not hasattr(lib, "axon_start_nrt_profile"):
            return None
        lib.axon_start_nrt_profile.argtypes = [
            ctypes.POINTER(ctypes.c_int64),
            ctypes.c_size_t,
        ]
        lib.axon_start_nrt_profile.restype = ctypes.c_int64
        lib.axon_stop_nrt_profile.argtypes = [ctypes.c_char_p]
        lib.axon_stop_nrt_profile.restype = ctypes.c_int64

        @contextlib.contextmanager
        def _hook(output_dir, device_ids):
            import jax

            jax.devices()
            if device_ids:
                ids = (ctypes.c_int64 * len(device_ids))(*device_ids)
                rc = lib.axon_start_nrt_profile(ids, len(device_ids))
            else:
                rc = lib.axon_start_nrt_profile(None, 0)
            if rc != 0:
                raise RuntimeError(f"axon_start_nrt_profile rc={rc}")
            try:
                yield
            finally:
                lib.axon_stop_nrt_profile(str(output_dir).encode())

        return _hook

    mod = types.ModuleType("antenv.axon_hooks")
    _holder = {"hook": _make_hook()}
    mod.set_axon_ntff_profile_hook = lambda h: _holder.__setitem__("hook", h)
    mod.get_axon_ntff_profile_hook = lambda: _holder["hook"]
    sys.modules["antenv.axon_hooks"] = mod
    try:
        import antenv

        antenv.axon_hooks = mod
    except ImportError:
        pass


def build_nc(enable_asserts=False):
    """Build + compile the per-core Bass program (identical on all 8 cores)."""
    from concourse import bacc, mybir, tile

    f32 = mybir.dt.float32
    bf16 = mybir.dt.bfloat16
    f8 = mybir.dt.float8e4
    AF = mybir.ActivationFunctionType
    ALU = mybir.AluOpType
    hp_dt = f8 if DR else bf16

    nc = bacc.Bacc(
        "TRN2",
        target_bir_lowering=False,
        debug=False,
        enable_asserts=enable_asserts,
        num_devices=N_CORES,
    )
    g8 = nc.dram_tensor("g8", [128, JC, ROWS], f8, kind="ExternalInput").ap()
    hT = nc.dram_tensor("hT", [D, N], bf16, kind="ExternalInput").ap()
    hc = nc.dram_tensor("hc", [128, JC, D], bf16, kind="ExternalInput").ap()
    W1 = nc.dram_tensor("W1", [D, HID], bf16, kind="ExternalInput").ap()
    b1 = nc.dram_tensor("b1", [HID, 1], f32, kind="ExternalInput").ap()
    W2 = nc.dram_tensor("W2", [HID, 1], bf16, kind="ExternalInput").ap()
    b2 = nc.dram_tensor("b2", [1, 1], f32, kind="ExternalInput").ap()
    # out stored p-major ([128, 8, D], partition-contiguous 4 KB lines; the
    # host inverts the permutation) — the natural (a p) d layout fragments
    # the final DMA into 512 B packets and costs ~10 us of tail
    out = nc.dram_tensor("out", [128, 8, D], f32, kind="ExternalOutput").ap()

    with tile.TileContext(nc) as tc:
        with (
            tc.tile_pool(name="const", bufs=1) as cpool,
            tc.tile_pool(name="big", bufs=1) as bigpool,
            tc.tile_pool(name="gbuf", bufs=6) as gpool,
            tc.tile_pool(name="hpbuf", bufs=16) as hpool,
            tc.tile_pool(name="outbuf", bufs=3) as opool,
            tc.tile_pool(name="small", bufs=2) as spool,
        ):
            # DMA-issue instructions cost ~700 ns of issuing-engine time each,
            # so they are spread across engines: consts+hc on gpsimd (SWDGE),
            # hT on the scalar engine's HW queue (scalar is idle early), G on
            # sync.  Few, big transfers.
            W1_sb = cpool.tile([D, HID], bf16)
            nc.gpsimd.dma_start(W1_sb[:], W1[:])
            b1_sb = cpool.tile([HID, 1], f32)
            nc.gpsimd.dma_start(b1_sb[:], b1[:])
            W2_sb = cpool.tile([HID, 1], bf16)
            nc.gpsimd.dma_start(W2_sb[:], W2[:])
            b2_sb = cpool.tile([1, 1], f32)
            nc.gpsimd.dma_start(b2_sb[:], b2[:])
            ones_row = cpool.tile([1, 128], f32)
            nc.vector.memset(ones_row[:], 1.0)

            NHCH = 2
            hT_sb = bigpool.tile([D, N], bf16)
            hc_sb = bigpool.tile([128, JC, D], bf16)
            for q in range(NHCH):
                sl = slice(q * (N // NHCH), (q + 1) * (N // NHCH))
                nc.scalar.dma_start(hT_sb[:, sl], hT[:, sl])
            for q in range(4):
                cl = slice(q * (JC // 4), (q + 1) * (JC // 4))
                nc.gpsimd.dma_start(hc_sb[:, cl, :], hc[:, cl, :])

            a_sb = bigpool.tile([HID, N], bf16)   # relu(h @ W1 + b1), bf16
            w_sb = cpool.tile([128, JC], f32)     # exp(e + ESHIFT) chunk-major
            # wtail[:, :, jc] = tail H' columns for chunk jc
            NT = 3 if DR else 2                   # [w_hi, w_lo, 1] / [w, 1]
            wtail = cpool.tile([128, NT, JC], hp_dt)
            nc.vector.memset(wtail[:, NT - 1, :], 1.0)

            with tc.tile_pool(name="ps_pre", bufs=2, space="PSUM") as ps_pre:
                # dummy matmuls on a zero tile: trip the PE HAM activity
                # monitor out of its cold clock before the real MLP arrives
                # (no input deps, runs during the first h DMA chunk)
                warm = cpool.tile([128, 512], bf16)
                nc.vector.memset(warm[:], 0.0)
                pwarm = ps_pre.tile([128, 512], f32, tag="pwarm")
                for _ in range(WARM):
                    nc.tensor.matmul(
                        pwarm[:], warm[:, 0:128], warm[:], start=True, stop=True
                    )

                # ebias = b2 + ESHIFT broadcast to 128 partitions via a K=1
                # matmul (avoids any slow single-partition ops)
                pb2 = ps_pre.tile([128, 1], f32, tag="pb2")
                nc.tensor.matmul(pb2[:], ones_row[:], b2_sb[:], start=True,
                                 stop=True)
                ebias_sb = cpool.tile([128, 1], f32)
                nc.vector.tensor_scalar_add(ebias_sb[:], pb2[:], ESHIFT)

                # MLP pipeline per quarter: z (4 blocks of 512) -> relu ->
                # e (16 chunk-stationary matmuls) -> exp -> w tails.  The
                # first quarter unblocks the main loop while later scores
                # still compute.
                pe = ps_pre.tile([128, JC], f32, tag="pe")
                QW = JC // 4
                for q in range(4):
                    for nb in range(4 * q, 4 * (q + 1)):
                        pz = ps_pre.tile([HID, 512], f32, tag="pz")
                        sl = slice(nb * 512, (nb + 1) * 512)
                        nc.tensor.matmul(
                            pz[:], W1_sb[:], hT_sb[:, sl], start=True, stop=True
                        )
                        nc.scalar.activation(
                            a_sb[:, sl], pz[:], AF.Relu, bias=b1_sb[:]
                        )
                    for c in range(q * QW, (q + 1) * QW):
                        nc.tensor.matmul(
                            pe[:, c : c + 1],
                            a_sb[:, c * 128 : (c + 1) * 128],
                            W2_sb[:],
                            start=True,
                            stop=True,
                        )
                    ql = slice(q * QW, (q + 1) * QW)
                    nc.scalar.activation(
                        w_sb[:, ql], pe[:, ql], AF.Exp, bias=ebias_sb[:]
                    )
                    nc.vector.tensor_copy(wtail[:, 0, ql], w_sb[:, ql])
                    if DR:
                        nc.vector.scalar_tensor_tensor(
                            wtail[:, 1, ql], w_sb[:, ql], 1.0, wtail[:, 0, ql],
                            op0=ALU.mult, op1=ALU.subtract,
                        )

            # Main accumulation: acc[it] [128, NCOL] += G_tile.T @ H'_chunk.
            with tc.tile_pool(name="ps_acc", bufs=8, space="PSUM") as ps_acc:
                accs = [
                    ps_acc.tile([128, NCOL], f32, tag="acc", name=f"acc{i}")
                    for i in range(8)
                ]

                def build_hp(jc):
                    # just-in-time H' chunk build
                    if DR:
                        # fp8 hi/lo pair for DoubleRow: built per chunk into
                        # half of a pair tile by the caller
                        raise NotImplementedError
                    hp = hpool.tile([128, NCOL], bf16, tag="hp",
                                    name=f"hp{jc}")
                    nc.vector.tensor_scalar_mul(
                        hp[:, 0:128], hc_sb[:, jc, :], w_sb[:, jc : jc + 1]
                    )
                    nc.vector.tensor_copy(hp[:, 128:130], wtail[:, :, jc])
                    return hp

                def build_hp2(pair):
                    # fp8 [wh_hi|wh_lo|w_hi|w_lo|1] x 2 chunks for DoubleRow
                    hp = hpool.tile([128, 2, NCOL], f8, tag="hp",
                                    name=f"hp{pair}")
                    for i in range(2):
                        jc = 2 * pair + i
                        nc.vector.tensor_scalar_mul(
                            hp[:, i, 0:128], hc_sb[:, jc, :],
                            w_sb[:, jc : jc + 1],
                        )
                        nc.vector.scalar_tensor_tensor(
                            hp[:, i, 128:256], hc_sb[:, jc, :],
                            w_sb[:, jc : jc + 1], hp[:, i, 0:128],
                            op0=ALU.mult, op1=ALU.subtract,
                        )
                        nc.vector.tensor_copy(hp[:, i, 256:259],
                                              wtail[:, :, jc])
                    return hp

                if DR:
                    import bass_rust

                    PM = bass_rust.MatmulPerfMode.DoubleRow
                    PAIRS_G = GRP // 2
                    for jg in range(JC // GRP):
                        last_g = jg == JC // GRP - 1
                        gt = gpool.tile([128, GRP, ROWS], f8, tag="gt")
                        nc.sync.dma_start(
                            gt[:], g8[:, jg * GRP : (jg + 1) * GRP, :]
                        )
                        if not last_g:
                            for pi in range(PAIRS_G):
                                hp = build_hp2(jg * PAIRS_G + pi)
                                for it in range(8):
                                    nc.tensor.matmul(
                                        accs[it][:],
                                        gt[:, 2 * pi : 2 * pi + 2,
                                           it * 128 : (it + 1) * 128],
                                        hp[:],
                                        start=(jg == 0 and pi == 0),
                                        stop=False,
                                        perf_mode=PM,
                                    )
                        else:
                            # last group it-major with staggered stop so each
                            # bank's epilogue overlaps remaining matmuls
                            hps = [build_hp2(jg * PAIRS_G + pi)
                                   for pi in range(PAIRS_G)]
                            for it in range(8):
                                for pi in range(PAIRS_G):
                                    nc.tensor.matmul(
                                        accs[it][:],
                                        gt[:, 2 * pi : 2 * pi + 2,
                                           it * 128 : (it + 1) * 128],
                                        hps[pi][:],
                                        start=False,
                                        stop=(pi == PAIRS_G - 1),
                                        perf_mode=PM,
                                    )
                else:
                    for jg in range(JC // GRP - 1):
                        gt = gpool.tile([128, GRP, ROWS], f8, tag="gt")
                        nc.sync.dma_start(
                            gt[:], g8[:, jg * GRP : (jg + 1) * GRP, :]
                        )
                        for jci in range(GRP):
                            jc = jg * GRP + jci
                            hp = build_hp(jc)
                            for it in range(8):
                                nc.tensor.matmul(
                                    accs[it][:],
                                    gt[:, jci, it * 128 : (it + 1) * 128],
                                    hp[:],
                                    start=(jc == 0),
                                    stop=False,
                                )
                    gt = gpool.tile([128, GRP, ROWS], f8, tag="gt",
                                    name="gt_last")
                    nc.sync.dma_start(gt[:], g8[:, JC - GRP :, :])
                    hps_last = [build_hp(JC - GRP + jci) for jci in range(GRP)]
                    for it in range(8):
                        for jci in range(GRP):
                            nc.tensor.matmul(
                                accs[it][:],
                                gt[:, jci, it * 128 : (it + 1) * 128],
                                hps_last[jci][:],
                                start=False,
                                stop=(jci == GRP - 1),
                            )

                # epilogue, fully per-bank: each bank's whole chain (tail
                # copy -> recip -> r -> scaled output) runs as soon as ITS
                # accumulator stops, overlapping the remaining banks' matmuls;
                # only bank 7's short chain + the out DMA trail the loop.
                # (one PSUM operand per DVE op; contiguous writes only)
                ot_all = opool.tile([128, 8, D], f32, tag="ot_all", bufs=1)
                for it in range(8):
                    tl = spool.tile([128, 2], f32, tag="tl", name=f"tl{it}",
                                    bufs=8)
                    nc.vector.tensor_copy(tl[:], accs[it][:, 128:130])
                    den = spool.tile([128, 1], f32, tag="den",
                                     name=f"den{it}", bufs=8)
                    nc.vector.tensor_scalar_add(den[:], tl[:, 0:1], 1e-30)
                    rc = spool.tile([128, 1], f32, tag="rc", name=f"rc{it}",
                                    bufs=8)
                    nc.vector.reciprocal(rc[:], den[:])
                    r1 = spool.tile([128, 1], f32, tag="r1", name=f"r1{it}",
                                    bufs=8)
                    nc.vector.tensor_mul(r1[:], rc[:], tl[:, 1:2])
                    nc.vector.tensor_scalar_mul(
                        ot_all[:, it, :], accs[it][:, 0:128], r1[:]
                    )
                nc.sync.dma_start(out[:], ot_all[:])

    nc.compile()
    return nc


def make_in_maps(graph_info, h, W1, b1, W2, b2):
    """Shard + lay out the full inputs for the 8 cores."""
    import ml_dtypes

    bf16 = ml_dtypes.bfloat16
    f8 = ml_dtypes.float8_e4m3fn

    # G (exact 0/1) as fp8, laid out [core][128 c, JC, ROWS] so the stationary
    # tile for (chunk jc, row block it) is g8[:, jc, it*128:(it+1)*128] and
    # every per-partition DMA line is contiguous
    g = np.asarray(graph_info, np.float32)
    G8 = g.astype(f8).reshape(N_CORES, ROWS, JC, 128).transpose(0, 3, 2, 1)
    h = np.asarray(h, np.float32)
    hTb = np.ascontiguousarray(h.T).astype(bf16)               # [D, N]
    hcb = np.ascontiguousarray(
        h.reshape(JC, 128, D).transpose(1, 0, 2)               # [128, JC, D]
    ).astype(bf16)
    W1b = np.asarray(W1, np.float32).astype(bf16)
    b1r = np.asarray(b1, np.float32).reshape(HID, 1)
    W2b = np.asarray(W2, np.float32).reshape(HID, 1).astype(bf16)
    b2r = np.asarray(b2, np.float32).reshape(1, 1)
    in_maps = []
    for c in range(N_CORES):
        in_maps.append(
            {
                "g8": np.ascontiguousarray(G8[c]),
                "hT": hTb,
                "hc": hcb,
                "W1": W1b,
                "b1": b1r,
                "W2": W2b,
                "b2": b2r,
            }
        )
    return in_maps


def kernel(graph_info, h, W1, b1, W2, b2):
    _install_axon_hooks_shim()
    from concourse.bass_utils import run_bass_kernel_spmd

    if "nc" not in _cache:
        _cache["nc"] = build_nc()
    nc = _cache["nc"]

    in_maps = make_in_maps(graph_info, h, W1, b1, W2, b2)
    res = run_bass_kernel_spmd(nc, in_maps, list(range(N_CORES)))
    # out is stored p-major [128, 8, D] per core; invert to row order
    return np.concatenate(
        [
            res.results[c]["out"].transpose(1, 0, 2).reshape(ROWS, D)
            for c in range(N_CORES)
        ],
        axis=0,
    )
